# revision 97
# baseline (speedup 1.0000x reference)
"""Trainium2 Bass kernel for nn_Attention_65317862638379.

Dense transformer block-attention with per-token geometric (rotation+translation)
transform. B=16, N=2048, DIM=1024, H=16, DH=64; attention over N/4=512 block
tokens of dim 256.

Sharding: data-parallel over batch, 2 batches per core, 8 cores, no collectives.
All matmuls run in bf16 on the PE (fp32 PSUM accumulation).

Layouts (per batch, per core):
  - Q/K: projection emitted per block-phase a (strided rhs) -> per-a PSUM
    [j=(head-pair,dh), I]; evicted with partition-offset copies into STACKED
    tiles qS/kS [(u,dh), I] per (head, a-pair) so the attention-score matmul
    contracts K=128 (2 steps) instead of K=64 (4 steps). Rotation applied
    in stacked layout via stream_shuffle (partition pair swap) + cosS/sinS.
  - V: a-split natural projection -> Vb [J, (h,a,dh)] block layout; fwd
    rotation on the free axis + translation. First (batch 0, a=0) group runs
    dk-outer over 8 PSUM banks so the PE pipelines with xT DMA arrival.
  - Attention sim[J, I] per head; softmax without max-subtraction; exp tiles
    pre-added pairwise on Pool so the partition-sum needs TWO ones-matmul
    accumulation steps (deps on two independent pool ops, not a chain).
  - Per-iteration emission [q-proj, sims(prev), k-proj, pv(prev), sums(prev)]
    keeps the static PE stream fed while ACT exps / pool pre-adds complete.
  - PV -> A [(a,dh), I]; normalize, inverse translate/rotate, write ao [j, t];
    final projection back to natural [t, e].
  - All weight/coefficient tensors are host-packed so each load is a single
    large DMA (HWDGE fixed cost is per-DMA).
"""

import numpy as np
import ml_dtypes

import concourse.bass as bass
import concourse.mybir as mybir
import concourse.tile as tile
from concourse.bass_utils import run_bass_kernel_spmd

BF16 = ml_dtypes.bfloat16

B, N, DIM, H, DH = 16, 2048, 1024, 16, 64
D_FLAT, D_ROT, NPAIR = 32, 32, 16
BLK = 4
NB = N // BLK          # 512 block tokens
DB = DH * BLK          # 256 block dim
NCORES = 8
B2 = B // NCORES       # batches per core
SCALE = float((DH * BLK) ** -0.5)  # 1/16, TAU=1.0

FP32 = mybir.dt.float32
BFD = mybir.dt.bfloat16

MULT = mybir.AluOpType.mult
ADD = mybir.AluOpType.add
SUB = mybir.AluOpType.subtract

_CACHE = {}


def _split_multi_waits(nc):
    """walrus codegen only supports one sync-wait per instruction; hoist
    extra waits onto preceding same-engine NoOps."""
    cnt = 0
    for f in nc.m.functions:
        for blk in f.blocks:
            insts = blk.instructions
            out = []
            for inst in insts:
                si = inst.sync_info
                if si is not None and si.on_wait and len(si.on_wait) > 1:
                    waits = list(si.on_wait)
                    for w in waits[:-1]:
                        cnt += 1
                        nop = mybir.InstNoOp(name=f"WSPLIT-{cnt}", ins=[], outs=[])
                        nop.engine = inst.engine
                        nop.sync_info = mybir.SyncInfo(on_wait=[w], on_update=[])
                        out.append(nop)
                    inst.sync_info = mybir.SyncInfo(
                        on_wait=[waits[-1]], on_update=list(si.on_update))
                out.append(inst)
            blk.instructions = out
    return cnt


def _build_nc():
    """Build the Bass graph (SPMD; same NEFF on all 8 cores)."""
    nc = bass.Bass(target_bir_lowering=False)

    # ---------------- DRAM parameters (per-core shapes) ----------------
    # host-packed layouts: leading dim 128 = SBUF partition
    xT_d = nc.dram_tensor("xT", [B2, DIM, N], BFD, kind="ExternalInput")
    wqkB_d = nc.dram_tensor("wqkB", [128, 8, 8, 2, 128], BFD, kind="ExternalInput")
    wvB_d = nc.dram_tensor("wvB", [128, 8, 1024], BFD, kind="ExternalInput")
    woutB_d = nc.dram_tensor("woutB", [128, 8, 1024], BFD, kind="ExternalInput")
    boutB_d = nc.dram_tensor("boutB", [128, DIM], FP32, kind="ExternalInput")
    cosS_d = nc.dram_tensor("cosS", [B2, 128, 2, NB], BFD, kind="ExternalInput")
    sinS_d = nc.dram_tensor("sinS", [B2, 128, 2, NB], BFD, kind="ExternalInput")
    cstN_d = nc.dram_tensor("cstN", [B2, BLK, 128, 4, 512], BFD, kind="ExternalInput")
    cosE_d = nc.dram_tensor("cosE", [B2, 128, 2, NB], BFD, kind="ExternalInput")
    sinE_d = nc.dram_tensor("sinE", [B2, 128, 2, NB], BFD, kind="ExternalInput")
    transB_d = nc.dram_tensor("transB", [B2, 128, 2, NB], BFD, kind="ExternalInput")

    out_d = nc.dram_tensor("out", [B2, N, DIM], BFD, kind="ExternalOutput")

    swap_mask = []
    for i in range(16):
        swap_mask += [2 * i + 1, 2 * i]

    from contextlib import ExitStack
    with ExitStack() as ctx:
        tc = ctx.enter_context(tile.TileContext(nc))
        ep = ctx.enter_context
        consts = ep(tc.tile_pool(name="consts", bufs=1))
        xT_pool = ep(tc.tile_pool(name="xT", bufs=1))
        wv_pool = ep(tc.tile_pool(name="wv", bufs=1))
        wqk_pool = ep(tc.tile_pool(name="wqk", bufs=2))
        wout_pool = ep(tc.tile_pool(name="wout", bufs=1))
        vb_pool = ep(tc.tile_pool(name="vb", bufs=1))
        qk_pool = ep(tc.tile_pool(name="qk", bufs=2))
        ao_pool = ep(tc.tile_pool(name="ao", bufs=1))
        coef_pool = ep(tc.tile_pool(name="coefs", bufs=1))
        cn_pool = ep(tc.tile_pool(name="cn", bufs=2))
        exp_pool = ep(tc.tile_pool(name="expt", bufs=8))
        sum_pool = ep(tc.tile_pool(name="sums", bufs=2))
        sum2_pool = ep(tc.tile_pool(name="sums2", bufs=2))
        tmps_pool = ep(tc.tile_pool(name="tmps", bufs=2))
        tmpb_pool = ep(tc.tile_pool(name="tmpb", bufs=2))
        tmpa_pool = ep(tc.tile_pool(name="tmpa", bufs=2))
        pve_pool = ep(tc.tile_pool(name="pve", bufs=3))
        oev_pool = ep(tc.tile_pool(name="oev", bufs=3))
        ps_pool = ep(tc.tile_pool(name="ps", bufs=8, space="PSUM"))
        if True:
            wv_sb = {}
            wqk_sb = {}
            wout_sb = {}
            bout_sb = {}
            ones_sb = consts.tile([128, 128], BFD)
            nc.vector.memset(ones_sb, 1.0)

            def load_first_weights():
                # wv in two half-DMAs on the sync queue, AFTER the xT loads
                # (DMA engines are serial: wv must not delay xT0)
                wv_sb[0] = wv_pool.tile([128, 8, 1024], BFD, tag="wv", name="wv")
                nc.sync.dma_start(out=wv_sb[0][:, :, 0:512],
                                  in_=wvB_d[:, :, 0:512])
                nc.sync.dma_start(out=wv_sb[0][:, :, 512:1024],
                                  in_=wvB_d[:, :, 512:1024])

            def load_late_weights():
                wout_sb[0] = wout_pool.tile([128, 8, 1024], BFD, tag="wout", name="wout")
                nc.sync.dma_start(out=wout_sb[0], in_=woutB_d[:, :, :])
                bout_sb[0] = consts.tile([128, DIM], FP32, tag="bout", name="bout")
                nc.sync.dma_start(out=bout_sb[0], in_=boutB_d[:, :])

            for b in range(B2):
                # wqk chunk-0 weights first: proj(0) dk-outer rides xT arrival
                wt0 = wqk_pool.tile([128, 8, 2, 128], BFD, tag="wqk",
                                    name="wt0")
                nc.sync.dma_start(out=wt0, in_=wqkB_d[:, 0])
                # ---- load xT for this batch: 8 d-chunk tiles [128, 2048] ----
                xT_sb = []
                for dk in range(8):
                    t = xT_pool.tile([128, N], BFD, tag=f"xT{dk}", name=f"xT{dk}")
                    deng = [nc.sync, nc.gpsimd][dk % 2]
                    deng.dma_start(
                        out=t, in_=xT_d[b, dk * 128:(dk + 1) * 128, :])
                    xT_sb.append(t)
                if b == 0:
                    load_first_weights()

                # ---- per-batch coefficient tiles ----
                def load_coefs():
                    cosS_sb = coef_pool.tile([128, 2, NB], BFD, tag="cosS", name="cosS")
                    sinS_sb = coef_pool.tile([128, 2, NB], BFD, tag="sinS", name="sinS")
                    nc.sync.dma_start(out=cosS_sb, in_=cosS_d[b])
                    nc.sync.dma_start(out=sinS_sb, in_=sinS_d[b])
                    cosE_sb = coef_pool.tile([128, 2, NB], BFD, tag="cosE", name="cosE")
                    sinE_sb = coef_pool.tile([128, 2, NB], BFD, tag="sinE", name="sinE")
                    transB_sb = coef_pool.tile([128, 2, NB], BFD, tag="transB", name="transB")
                    nc.sync.dma_start(out=cosE_sb, in_=cosE_d[b])
                    nc.sync.dma_start(out=sinE_sb, in_=sinE_d[b])
                    nc.sync.dma_start(out=transB_sb, in_=transB_d[b])
                    return cosS_sb, sinS_sb, cosE_sb, sinE_sb, transB_sb

                # ================= V projection (a-split, natural) ==========
                # Vb store: per J-chunk tile [128, (h,a,dh)=4096]
                vb_sb = []
                for jc in range(4):
                    vt = vb_pool.tile([128, H * BLK * DH], BFD, tag=f"vb{jc}")
                    vb_sb.append(vt)

                def evict_v(ps, a, c, jsl, cst_a):
                    """PSUM [128 j, 512 vcols] -> vb (flat copy + rotated+
                    translated rot half)."""
                    cn_v = cst_a[:, c, 0:128].rearrange("p (h i) -> p h i", h=8)
                    sn_v = cst_a[:, c, 128:256].rearrange("p (h i) -> p h i", h=8)
                    tn_v = cst_a[:, c, 256:512].rearrange(
                        "p (h i t) -> p h i t", h=8, i=16, t=2)
                    pvr = tmps_pool.tile([128, 512], BFD, tag="pvr")
                    pv = pvr.rearrange(
                        "p (h half i t) -> p h half i t", h=8, half=2, i=16, t=2)
                    psv = ps.rearrange(
                        "p (h half i t) -> p h half i t", h=8, half=2, i=16, t=2)
                    nc.scalar.copy(out=pv[:, :, 1], in_=psv[:, :, 1])
                    x0 = pv[:, :, 1, :, 0]
                    x1 = pv[:, :, 1, :, 1]
                    dst = vb_sb[c].rearrange(
                        "p (h a half i t) -> p h a half i t",
                        h=16, a=4, half=2, i=16, t=2)
                    hlo, hhi = jsl * 8, (jsl + 1) * 8
                    dflat = dst[:, hlo:hhi, a, 0]
                    de = dst[:, hlo:hhi, a, 1, :, 0]
                    do = dst[:, hlo:hhi, a, 1, :, 1]
                    nc.scalar.copy(out=dflat, in_=psv[:, :, 0])
                    t0 = tmps_pool.tile([128, 8, 16], BFD, tag="t0")
                    t1 = tmps_pool.tile([128, 8, 16], BFD, tag="t1")
                    t4 = tmps_pool.tile([128, 8, 16], BFD, tag="t4")
                    veng = nc.vector if (a * 4 + c) % 3 else nc.gpsimd
                    veng.tensor_tensor(t0, x0, cn_v, MULT)
                    veng.tensor_tensor(t1, x1, sn_v, MULT)
                    veng.tensor_tensor(t4, t0, t1, SUB)
                    # even_rot = x0 cos - x1 sin + c*trans_even
                    veng.tensor_tensor(de, t4, tn_v[:, :, :, 0], ADD)
                    t2 = tmps_pool.tile([128, 8, 16], BFD, tag="t2")
                    t3 = tmps_pool.tile([128, 8, 16], BFD, tag="t3")
                    t5 = tmps_pool.tile([128, 8, 16], BFD, tag="t5")
                    veng.tensor_tensor(t2, x0, sn_v, MULT)
                    veng.tensor_tensor(t3, x1, cn_v, MULT)
                    veng.tensor_tensor(t5, t2, t3, ADD)
                    veng.tensor_tensor(do, t5, tn_v[:, :, :, 1], ADD)

                def emit_v_phase():
                    for a in range(BLK):
                        cst_a = cn_pool.tile([128, 4, 512], BFD, tag="cstN",
                                             name="cst_a")
                        nc.sync.dma_start(out=cst_a, in_=cstN_d[b, a])
                        xv = [xT_sb[dk].rearrange(
                            "p (c j a) -> p c j a", c=4, j=128, a=4)
                            for dk in range(8)]
                        for c in range(4):
                            for jsl in range(2):
                                ps = ps_pool.tile([128, 512], FP32, tag="ps",
                                                  name="psv")
                                for dk in range(8):
                                    nc.tensor.matmul(
                                        ps, xv[dk][:, c, :, a],
                                        wv_sb[0][:, dk, jsl * 512:(jsl + 1) * 512],
                                        start=(dk == 0), stop=(dk == 7))
                                evict_v(ps, a, c, jsl, cst_a)

                # ================= Q/K pairs + attention ====================
                xa = [xT_sb[dk].rearrange("p (i a) -> p i a", a=4)
                      for dk in range(8)]
                coef_tiles = {}

                def emit_rot(which, qk_tiles):
                    # rotation in stacked layout (in-place, [128, 1024] ops)
                    cosS_sb, sinS_sb = coef_tiles["cosS"], coef_tiles["sinS"]
                    for hh in range(2):
                        ft = qk_tiles[(which, hh)]
                        shuf = tmpb_pool.tile([128, 2, NB], BFD, tag="shuf")
                        nc.vector.stream_shuffle(shuf, ft, swap_mask)
                        nc.vector.tensor_tensor(ft, ft, cosS_sb, MULT)
                        nc.vector.tensor_tensor(shuf, shuf, sinS_sb, MULT)
                        nc.vector.tensor_tensor(ft, ft, shuf, ADD)

                def emit_proj_w(c2, wi, which, wt, qk_tiles):
                    """Project one of q/k for chunk c2 (2 heads) into STACKED
                    tiles [(u,dh), (cpair, I)] per head, then rotate."""
                    st = {}
                    for hh in range(2):
                        ft = qk_pool.tile([128, 2, NB], BFD,
                                          tag=f"{which}S{hh}", name="ft")
                        st[hh] = ft
                        qk_tiles[(which, hh)] = ft
                    for a in range(4):
                        ps = ps_pool.tile([128, 512], FP32, tag="ps")
                        for dk in range(8):
                            nc.tensor.matmul(
                                ps, wt[:, dk, wi, :], xa[dk][:, :, a],
                                start=(dk == 0), stop=(dk == 7))
                        for hh in range(2):
                            nc.scalar.copy(
                                out=st[hh][(a % 2) * 64:(a % 2 + 1) * 64,
                                           a // 2, :],
                                in_=ps[hh * 64:(hh + 1) * 64, :])
                    emit_rot(which, qk_tiles)

                def emit_proj0_mm(wt, qk_tiles):
                    """proj(0), q AND k, dk-outer over 8 PSUM chains so the
                    batch-0 PE rides the xT DMA arrival. Rotation deferred
                    (cosS not yet loaded)."""
                    st = {}
                    for wi, which in ((0, "q"), (1, "k")):
                        for hh in range(2):
                            ft = qk_pool.tile([128, 2, NB], BFD,
                                              tag=f"{which}S{hh}", name="ft")
                            st[(wi, hh)] = ft
                            qk_tiles[(which, hh)] = ft
                    pss = [[ps_pool.tile([128, 512], FP32, tag="ps",
                                         name=f"psp{wi}{a}")
                            for a in range(4)] for wi in range(2)]
                    dk_order = [1, 0, 3, 2, 5, 4, 7, 6]  # xT arrival order
                    for di, dk in enumerate(dk_order):
                        for wi in range(2):
                            for a in range(4):
                                nc.tensor.matmul(
                                    pss[wi][a], wt[:, dk, wi, :],
                                    xa[dk][:, :, a],
                                    start=(di == 0), stop=(di == 7))
                    for wi in range(2):
                        for a in range(4):
                            for hh in range(2):
                                nc.scalar.copy(
                                    out=st[(wi, hh)][
                                        (a % 2) * 64:(a % 2 + 1) * 64,
                                        a // 2, :],
                                    in_=pss[wi][a][hh * 64:(hh + 1) * 64, :])

                def emit_sims(c2, qk_tiles):
                    """sims + exps for both heads; pre-adds on Pool."""
                    expts = {}
                    sums = {}
                    for hh in range(2):
                        for Jc in range(4):
                            sim_ps = ps_pool.tile([128, 512], FP32, tag="ps",
                                                  name="sim")
                            for cpair in range(2):
                                nc.tensor.matmul(
                                    sim_ps,
                                    qk_tiles[("k", hh)][
                                        :, cpair, Jc * 128:(Jc + 1) * 128],
                                    qk_tiles[("q", hh)][:, cpair, :],
                                    start=(cpair == 0), stop=(cpair == 1))
                            et = exp_pool.tile([128, 512], BFD, tag="expt")
                            nc.scalar.activation(
                                out=et, in_=sim_ps,
                                func=mybir.ActivationFunctionType.Exp,
                                scale=SCALE)
                            expts[(hh, Jc)] = et
                        s01 = sum_pool.tile([128, 512], BFD, tag="s01")
                        s23 = sum2_pool.tile([128, 512], BFD, tag="s23")
                        nc.gpsimd.tensor_tensor(
                            s01, expts[(hh, 0)], expts[(hh, 1)], ADD)
                        nc.gpsimd.tensor_tensor(
                            s23, expts[(hh, 2)], expts[(hh, 3)], ADD)
                        sums[hh] = (s01, s23)
                    return expts, sums

                def emit_attn_tail(c2, expts, sums, mid_hook=None):
                    cosE_sb = coef_tiles["cosE"]
                    sinE_sb = coef_tiles["sinE"]
                    transB_sb = coef_tiles["transB"]
                    mid_result = None
                    # PV + ACT eviction to SBUF (frees PSUM early); placed
                    # BEFORE the denominator matmuls so the slow Pool pre-add
                    # chain has PV's PE time to complete
                    pves = {}
                    for hh in range(2):
                        h = 2 * c2 + hh
                        for cp in range(2):
                            pv_ps = ps_pool.tile([128, 512], FP32, tag="ps",
                                                 name=f"pv{cp}")
                            for Jc in range(4):
                                lhsT = vb_sb[Jc].rearrange(
                                    "p (h a d) -> p h a d", h=16, a=4, d=64)[
                                        :, h, 2 * cp:2 * cp + 2, :]
                                nc.tensor.matmul(
                                    pv_ps, lhsT, expts[(hh, Jc)],
                                    start=(Jc == 0), stop=(Jc == 3))
                            pve = pve_pool.tile([128, 512], BFD, tag="pve")
                            nc.scalar.copy(out=pve, in_=pv_ps)
                            pves[(hh, cp)] = pve
                    if mid_hook is not None:
                        mid_result = mid_hook()
                    # denominator matmuls + Ln/exp reciprocal
                    rsums_t = {}
                    for hh in range(2):
                        sums_ps = ps_pool.tile([128, 512], FP32, tag="ps",
                                               name="sums")
                        nc.tensor.matmul(sums_ps, ones_sb, sums[hh][0],
                                         start=True, stop=False)
                        nc.tensor.matmul(sums_ps, ones_sb, sums[hh][1],
                                         start=False, stop=True)
                        nc.scalar.activation(
                            out=sums_ps, in_=sums_ps,
                            func=mybir.ActivationFunctionType.Ln)
                        rsums = tmpa_pool.tile([128, 512], BFD, tag="rsums")
                        nc.scalar.activation(
                            out=rsums, in_=sums_ps,
                            func=mybir.ActivationFunctionType.Exp, scale=-1.0)
                        rsums_t[hh] = rsums
                    # normalize + inverse transform (DVE, u2-mul on Pool)
                    for hh in range(2):
                        plo, phi = hh * 64, (hh + 1) * 64
                        for cp in range(2):
                            asb = tmpa_pool.tile([128, 512], BFD, tag="asb")
                            nc.vector.tensor_tensor(
                                asb, pves[(hh, cp)], rsums_t[hh], MULT)
                            nc.vector.tensor_tensor(
                                asb, asb, transB_sb[:, cp, :], SUB)
                            shf = tmpa_pool.tile([128, 512], BFD, tag="shf")
                            nc.vector.stream_shuffle(shf, asb, swap_mask)
                            nc.vector.tensor_tensor(
                                asb, asb, cosE_sb[:, cp, :], MULT)
                            nc.gpsimd.tensor_tensor(
                                shf, shf, sinE_sb[:, cp, :], MULT)
                            aov = ao_sb[c2].rearrange("p (a i) -> p a i", a=4)
                            for ap2 in range(2):
                                nc.vector.tensor_tensor(
                                    aov[plo:phi, 2 * cp + ap2, :],
                                    asb[ap2 * 64:(ap2 + 1) * 64, :],
                                    shf[ap2 * 64:(ap2 + 1) * 64, :],
                                    ADD)
                    return mid_result

                # ---- proj(0): combined q+k dk-outer emitted BEFORE the V
                # phase so the batch-0 PE rides the xT DMA arrival
                qk0 = {}
                emit_proj0_mm(wt0, qk0)
                emit_v_phase()
                cosS_sb, sinS_sb, cosE_sb, sinE_sb, transB_sb = load_coefs()
                coef_tiles.update(cosS=cosS_sb, sinS=sinS_sb, cosE=cosE_sb,
                                  sinE=sinE_sb, transB=transB_sb)
                if b == 0:
                    load_late_weights()
                ao_sb = []
                for c2 in range(8):
                    at = ao_pool.tile([128, N], BFD, tag=f"ao{c2}")
                    ao_sb.append(at)
                emit_rot("q", qk0)
                emit_rot("k", qk0)

                # -- output projection helpers (NWAVE warmup is emitted inside
                # the last attention iteration to pad its PE gaps)
                out_v = out_d[b].rearrange("(i a) e -> i a e", a=4)
                groups = [(a, cI, esl) for a in range(4) for cI in range(4)
                          for esl in range(2)]
                NWAVE = 2

                def outp_evict(ps, a, cI, esl):
                    oev = oev_pool.tile([128, 512], BFD, tag="oev")
                    nc.vector.tensor_tensor(
                        oev, ps, bout_sb[0][:, esl * 512:(esl + 1) * 512], ADD)
                    nc.sync.dma_start(
                        out=out_v[cI * 128:(cI + 1) * 128, a,
                                  esl * 512:(esl + 1) * 512],
                        in_=oev)

                def outp_warmup():
                    wave = []
                    for gi in range(NWAVE):
                        a, cI, esl = groups[gi]
                        ps = ps_pool.tile([128, 512], FP32, tag="ps",
                                          name=f"fw{gi}")
                        for jc in range(6):
                            nc.tensor.matmul(
                                ps,
                                ao_sb[jc][:, a * 512 + cI * 128:
                                          a * 512 + (cI + 1) * 128],
                                wout_sb[0][:, jc, esl * 512:(esl + 1) * 512],
                                start=(jc == 0), stop=False)
                        wave.append(ps)
                    return wave

                qk_prev = qk0
                prev_c2 = 0
                wave = None
                for c2 in range(1, 9):
                    qk_cur = {}
                    if c2 < 8:
                        wt = wqk_pool.tile([128, 8, 2, 128], BFD,
                                           tag="wqk", name="wt")
                        nc.sync.dma_start(out=wt, in_=wqkB_d[:, c2])
                        emit_proj_w(c2, 0, "q", wt, qk_cur)
                    expts, sums = emit_sims(prev_c2, qk_prev)
                    if c2 < 8:
                        emit_proj_w(c2, 1, "k", wt, qk_cur)
                        emit_attn_tail(prev_c2, expts, sums)
                    else:
                        wave = emit_attn_tail(prev_c2, expts, sums,
                                              mid_hook=outp_warmup)
                    qk_prev = qk_cur
                    prev_c2 = c2

                # ================= output projection ========================
                for gi in range(NWAVE):
                    a, cI, esl = groups[gi]
                    for jc in (6, 7):
                        nc.tensor.matmul(
                            wave[gi],
                            ao_sb[jc][:, a * 512 + cI * 128:
                                      a * 512 + (cI + 1) * 128],
                            wout_sb[0][:, jc, esl * 512:(esl + 1) * 512],
                            start=False, stop=(jc == 7))
                    outp_evict(wave[gi], a, cI, esl)
                for gi in range(NWAVE, len(groups)):
                    a, cI, esl = groups[gi]
                    ps = ps_pool.tile([128, 512], FP32, tag="ps")
                    for jc in range(8):
                        nc.tensor.matmul(
                            ps,
                            ao_sb[jc][:, a * 512 + cI * 128:
                                      a * 512 + (cI + 1) * 128],
                            wout_sb[0][:, jc, esl * 512:(esl + 1) * 512],
                            start=(jc == 0), stop=(jc == 7))
                    outp_evict(ps, a, cI, esl)
    _split_multi_waits(nc)
    return nc


def _host_prep(x, angles, trans, W_qkv, W_out, b_out, trans_coeff):
    """Build all per-core input arrays (layout/dtype staging + cos/sin coeffs)."""
    c = float(np.asarray(trans_coeff).reshape(-1)[0])
    cos = np.cos(angles).astype(np.float32)   # [B, N, 16]
    sin = np.sin(angles).astype(np.float32)

    xT = np.ascontiguousarray(x.transpose(0, 2, 1)).astype(BF16)       # [B, DIM, N]
    wqkvT = np.ascontiguousarray(np.asarray(W_qkv).T).astype(np.float32)  # [DIM, 3HDH]
    # wqkB[p, c2, dk, w, j] = wqkvT[dk*128+p, w*1024 + c2*128 + j]
    wqkB = np.ascontiguousarray(
        wqkvT[:, :2048].reshape(8, 128, 2, 8, 128)
        .transpose(1, 3, 0, 2, 4)).astype(BF16)
    # wvB[p, dk, j] = wqkvT[dk*128+p, 2048+j]
    wvB = np.ascontiguousarray(
        wqkvT[:, 2048:].reshape(8, 128, 1024).transpose(1, 0, 2)).astype(BF16)
    woutT = np.asarray(W_out).T.astype(np.float32)                     # [DIM, DIM]
    woutB = np.ascontiguousarray(
        woutT.reshape(8, 128, 1024).transpose(1, 0, 2)).astype(BF16)
    boutB = np.ascontiguousarray(
        np.broadcast_to(np.asarray(b_out)[None, :], (128, DIM))).astype(np.float32)

    dh = np.arange(DH)
    pair_idx = np.clip((dh - D_FLAT) // 2, 0, NPAIR - 1)               # [64]
    is_rot = dh >= D_FLAT
    is_odd = ((dh - D_FLAT) % 2 == 1) & is_rot

    I = np.arange(NB)

    # ---- cosS/sinS [B, 128, 2, NB]: rows = (u, dh); fwd rotation in stacked
    # layout: token t = 4I + 2*cpair + u
    sgn = np.where(is_rot, np.where(is_odd, 1.0, -1.0), 0.0)
    cosS = np.empty((B, 128, 2, NB), np.float32)
    sinS = np.empty((B, 128, 2, NB), np.float32)
    for cpair in range(2):
        for u in range(2):
            t_idx = 4 * I + 2 * cpair + u
            cc = cos[:, t_idx, :][:, :, pair_idx].transpose(0, 2, 1)   # [B,64,NB]
            ss = sin[:, t_idx, :][:, :, pair_idx].transpose(0, 2, 1)
            cosS[:, u * 64:(u + 1) * 64, cpair, :] = np.where(
                is_rot[None, :, None], cc, 1.0)
            sinS[:, u * 64:(u + 1) * 64, cpair, :] = ss * sgn[None, :, None]
    cosS = cosS.astype(BF16)
    sinS = sinS.astype(BF16)

    # ---- cstN [B, BLK, 128, 4, 512] for V: rows = J%128, c = J//128,
    # cols (h=8, i=16) x {cos, sin, c*trans}
    J = np.arange(NB)
    cstN = np.empty((B, BLK, NB, 512), np.float32)
    for a in range(BLK):
        t_idx = 4 * J + a
        cstN[:, a, :, 0:128] = np.tile(cos[:, t_idx, :], (1, 1, 8))
        cstN[:, a, :, 128:256] = np.tile(sin[:, t_idx, :], (1, 1, 8))
        cstN[:, a, :, 256:512] = np.tile(c * np.asarray(trans)[:, t_idx, :], (1, 1, 8))
    cstN = np.ascontiguousarray(
        cstN.reshape(B, BLK, 4, 128, 512).transpose(0, 1, 3, 2, 4)).astype(BF16)

    # ---- inverse coeffs [B, 128, 2, NB]: rows = (a2, dh); t = 4I + 2*cp + a2
    cosE = np.empty((B, 128, 2, NB), np.float32)
    sinE = np.empty((B, 128, 2, NB), np.float32)
    transB = np.zeros((B, 128, 2, NB), np.float32)
    sgnE = np.where(is_rot, np.where(is_odd, -1.0, 1.0), 0.0)
    for cp in range(2):
        for a2 in range(2):
            t_idx = 4 * I + 2 * cp + a2
            cc = cos[:, t_idx, :][:, :, pair_idx].transpose(0, 2, 1)   # [B,64,NB]
            ss = sin[:, t_idx, :][:, :, pair_idx].transpose(0, 2, 1)
            cosE[:, a2 * 64:(a2 + 1) * 64, cp, :] = np.where(
                is_rot[None, :, None], cc, 1.0)
            sinE[:, a2 * 64:(a2 + 1) * 64, cp, :] = ss * sgnE[None, :, None]
            tb = c * np.asarray(trans)[:, t_idx, :].transpose(0, 2, 1)  # [B,32,NB]
            transB[:, a2 * 64 + D_FLAT:(a2 + 1) * 64, cp, :] = tb
    cosE = cosE.astype(BF16)
    sinE = sinE.astype(BF16)

    return dict(xT=xT, wqkB=wqkB, wvB=wvB, woutB=woutB, boutB=boutB,
                cosS=cosS, sinS=sinS, cstN=cstN,
                cosE=cosE, sinE=sinE, transB=transB.astype(BF16))


def kernel(x, angles, trans, W_qkv, W_out, b_out, trans_coeff, _profile=False):
    x = np.asarray(x)
    angles = np.asarray(angles)
    trans = np.asarray(trans)
    arrs = _host_prep(x, angles, trans, W_qkv, W_out, b_out, trans_coeff)
    if "nc" not in _CACHE:
        _CACHE["nc"] = _build_nc()
    nc = _CACHE["nc"]

    in_maps = []
    for core in range(NCORES):
        bsl = slice(core * B2, (core + 1) * B2)
        m = dict(
            xT=np.ascontiguousarray(arrs["xT"][bsl]),
            wqkB=arrs["wqkB"], wvB=arrs["wvB"], woutB=arrs["woutB"],
            boutB=arrs["boutB"],
            cosS=np.ascontiguousarray(arrs["cosS"][bsl]),
            sinS=np.ascontiguousarray(arrs["sinS"][bsl]),
            cstN=np.ascontiguousarray(arrs["cstN"][bsl]),
            cosE=np.ascontiguousarray(arrs["cosE"][bsl]),
            sinE=np.ascontiguousarray(arrs["sinE"][bsl]),
            transB=np.ascontiguousarray(arrs["transB"][bsl]),
        )
        in_maps.append(m)

    res = run_bass_kernel_spmd(nc, in_maps, core_ids=list(range(NCORES)),
                               trace=_profile)
    out = np.concatenate([r["out"] for r in res.results], axis=0).astype(np.float32)
    if _profile:
        _CACHE["last_exec_time_ns"] = res.exec_time_ns
        _CACHE["last_trace"] = res.instructions_and_trace
    return out


# revision 102
# speedup vs baseline: 1.0044x; 1.0044x over previous
"""Trainium2 Bass kernel for nn_Attention_65317862638379.

Dense transformer block-attention with per-token geometric (rotation+translation)
transform. B=16, N=2048, DIM=1024, H=16, DH=64; attention over N/4=512 block
tokens of dim 256.

Sharding: data-parallel over batch, 2 batches per core, 8 cores, no collectives.
All matmuls run in bf16 on the PE (fp32 PSUM accumulation).

Layouts (per batch, per core):
  - Q/K: projection emitted per block-phase a (strided rhs) -> per-a PSUM
    [j=(head-pair,dh), I]; evicted with partition-offset copies into STACKED
    tiles qS/kS [(u,dh), I] per (head, a-pair) so the attention-score matmul
    contracts K=128 (2 steps) instead of K=64 (4 steps). Rotation applied
    in stacked layout via stream_shuffle (partition pair swap) + cosS/sinS.
  - V: a-split natural projection -> Vb [J, (h,a,dh)] block layout; fwd
    rotation on the free axis + translation. First (batch 0, a=0) group runs
    dk-outer over 8 PSUM banks so the PE pipelines with xT DMA arrival.
  - Attention sim[J, I] per head; softmax without max-subtraction; exp tiles
    pre-added pairwise on Pool so the partition-sum needs TWO ones-matmul
    accumulation steps (deps on two independent pool ops, not a chain).
  - Per-iteration emission [q-proj, sims(prev), k-proj, pv(prev), sums(prev)]
    keeps the static PE stream fed while ACT exps / pool pre-adds complete.
  - PV -> A [(a,dh), I]; normalize, inverse translate/rotate, write ao [j, t];
    final projection back to natural [t, e].
  - All weight/coefficient tensors are host-packed so each load is a single
    large DMA (HWDGE fixed cost is per-DMA).
"""

import numpy as np
import ml_dtypes

import concourse.bass as bass
import concourse.mybir as mybir
import concourse.tile as tile
from concourse.bass_utils import run_bass_kernel_spmd

BF16 = ml_dtypes.bfloat16

B, N, DIM, H, DH = 16, 2048, 1024, 16, 64
D_FLAT, D_ROT, NPAIR = 32, 32, 16
BLK = 4
NB = N // BLK          # 512 block tokens
DB = DH * BLK          # 256 block dim
NCORES = 8
B2 = B // NCORES       # batches per core
SCALE = float((DH * BLK) ** -0.5)  # 1/16, TAU=1.0

FP32 = mybir.dt.float32
BFD = mybir.dt.bfloat16

MULT = mybir.AluOpType.mult
ADD = mybir.AluOpType.add
SUB = mybir.AluOpType.subtract

_CACHE = {}


def _split_multi_waits(nc):
    """walrus codegen only supports one sync-wait per instruction; hoist
    extra waits onto preceding same-engine NoOps."""
    cnt = 0
    for f in nc.m.functions:
        for blk in f.blocks:
            insts = blk.instructions
            out = []
            for inst in insts:
                si = inst.sync_info
                if si is not None and si.on_wait and len(si.on_wait) > 1:
                    waits = list(si.on_wait)
                    for w in waits[:-1]:
                        cnt += 1
                        nop = mybir.InstNoOp(name=f"WSPLIT-{cnt}", ins=[], outs=[])
                        nop.engine = inst.engine
                        nop.sync_info = mybir.SyncInfo(on_wait=[w], on_update=[])
                        out.append(nop)
                    inst.sync_info = mybir.SyncInfo(
                        on_wait=[waits[-1]], on_update=list(si.on_update))
                out.append(inst)
            blk.instructions = out
    return cnt


def _build_nc():
    """Build the Bass graph (SPMD; same NEFF on all 8 cores)."""
    nc = bass.Bass(target_bir_lowering=False)

    # ---------------- DRAM parameters (per-core shapes) ----------------
    # host-packed layouts: leading dim 128 = SBUF partition
    xT_d = nc.dram_tensor("xT", [B2, DIM, N], BFD, kind="ExternalInput")
    wqkB_d = nc.dram_tensor("wqkB", [128, 8, 8, 2, 128], BFD, kind="ExternalInput")
    wvB_d = nc.dram_tensor("wvB", [128, 8, 1024], BFD, kind="ExternalInput")
    woutB_d = nc.dram_tensor("woutB", [128, 8, 1024], BFD, kind="ExternalInput")
    boutB_d = nc.dram_tensor("boutB", [128, DIM], FP32, kind="ExternalInput")
    cosS_d = nc.dram_tensor("cosS", [B2, 128, 2, NB], BFD, kind="ExternalInput")
    sinS_d = nc.dram_tensor("sinS", [B2, 128, 2, NB], BFD, kind="ExternalInput")
    cstN_d = nc.dram_tensor("cstN", [B2, BLK, 128, 4, 512], BFD, kind="ExternalInput")
    cosE_d = nc.dram_tensor("cosE", [B2, 128, 2, NB], BFD, kind="ExternalInput")
    sinE_d = nc.dram_tensor("sinE", [B2, 128, 2, NB], BFD, kind="ExternalInput")
    transB_d = nc.dram_tensor("transB", [B2, 128, 2, NB], BFD, kind="ExternalInput")

    out_d = nc.dram_tensor("out", [B2, N, DIM], BFD, kind="ExternalOutput")

    swap_mask = []
    for i in range(16):
        swap_mask += [2 * i + 1, 2 * i]

    from contextlib import ExitStack
    with ExitStack() as ctx:
        tc = ctx.enter_context(tile.TileContext(nc))
        ep = ctx.enter_context
        consts = ep(tc.tile_pool(name="consts", bufs=1))
        xT_pool = ep(tc.tile_pool(name="xT", bufs=1))
        wv_pool = ep(tc.tile_pool(name="wv", bufs=1))
        wqk_pool = ep(tc.tile_pool(name="wqk", bufs=2))
        wout_pool = ep(tc.tile_pool(name="wout", bufs=1))
        vb_pool = ep(tc.tile_pool(name="vb", bufs=1))
        qk_pool = ep(tc.tile_pool(name="qk", bufs=2))
        ao_pool = ep(tc.tile_pool(name="ao", bufs=1))
        coef_pool = ep(tc.tile_pool(name="coefs", bufs=1))
        cn_pool = ep(tc.tile_pool(name="cn", bufs=2))
        exp_pool = ep(tc.tile_pool(name="expt", bufs=8))
        sum_pool = ep(tc.tile_pool(name="sums", bufs=2))
        sum2_pool = ep(tc.tile_pool(name="sums2", bufs=2))
        tmps_pool = ep(tc.tile_pool(name="tmps", bufs=2))
        tmpb_pool = ep(tc.tile_pool(name="tmpb", bufs=2))
        tmpa_pool = ep(tc.tile_pool(name="tmpa", bufs=2))
        pve_pool = ep(tc.tile_pool(name="pve", bufs=3))
        oev_pool = ep(tc.tile_pool(name="oev", bufs=3))
        ps_pool = ep(tc.tile_pool(name="ps", bufs=8, space="PSUM"))
        if True:
            wv_sb = {}
            wqk_sb = {}
            wout_sb = {}
            bout_sb = {}
            ones_sb = consts.tile([128, 128], BFD)
            nc.vector.memset(ones_sb, 1.0)

            def load_first_weights():
                # wv in two half-DMAs on the sync queue, AFTER the xT loads
                # (DMA engines are serial: wv must not delay xT0)
                wv_sb[0] = wv_pool.tile([128, 8, 1024], BFD, tag="wv", name="wv")
                nc.sync.dma_start(out=wv_sb[0][:, :, 0:512],
                                  in_=wvB_d[:, :, 0:512])
                nc.sync.dma_start(out=wv_sb[0][:, :, 512:1024],
                                  in_=wvB_d[:, :, 512:1024])

            def load_late_weights():
                wout_sb[0] = wout_pool.tile([128, 8, 1024], BFD, tag="wout", name="wout")
                nc.sync.dma_start(out=wout_sb[0], in_=woutB_d[:, :, :])
                bout_sb[0] = consts.tile([128, DIM], FP32, tag="bout", name="bout")
                nc.sync.dma_start(out=bout_sb[0], in_=boutB_d[:, :])

            for b in range(B2):
                # wqk chunk-0 weights first: proj(0) dk-outer rides xT arrival
                wt0 = wqk_pool.tile([128, 8, 2, 128], BFD, tag="wqk",
                                    name="wt0")
                # split: first half covers dk 0-3, enough for the first
                # dk-outer rounds while the rest streams in
                nc.sync.dma_start(out=wt0[:, 0:4], in_=wqkB_d[:, 0, 0:4])
                nc.sync.dma_start(out=wt0[:, 4:8], in_=wqkB_d[:, 0, 4:8])
                # ---- load xT for this batch: 8 d-chunk tiles [128, 2048] ----
                xT_sb = []
                for dk in range(8):
                    t = xT_pool.tile([128, N], BFD, tag=f"xT{dk}", name=f"xT{dk}")
                    deng = [nc.sync, nc.gpsimd][dk % 2]
                    deng.dma_start(
                        out=t, in_=xT_d[b, dk * 128:(dk + 1) * 128, :])
                    xT_sb.append(t)
                if b == 0:
                    load_first_weights()

                # ---- per-batch coefficient tiles ----
                def load_coefs():
                    cosS_sb = coef_pool.tile([128, 2, NB], BFD, tag="cosS", name="cosS")
                    sinS_sb = coef_pool.tile([128, 2, NB], BFD, tag="sinS", name="sinS")
                    nc.sync.dma_start(out=cosS_sb, in_=cosS_d[b])
                    nc.sync.dma_start(out=sinS_sb, in_=sinS_d[b])
                    cosE_sb = coef_pool.tile([128, 2, NB], BFD, tag="cosE", name="cosE")
                    sinE_sb = coef_pool.tile([128, 2, NB], BFD, tag="sinE", name="sinE")
                    transB_sb = coef_pool.tile([128, 2, NB], BFD, tag="transB", name="transB")
                    nc.sync.dma_start(out=cosE_sb, in_=cosE_d[b])
                    nc.sync.dma_start(out=sinE_sb, in_=sinE_d[b])
                    nc.sync.dma_start(out=transB_sb, in_=transB_d[b])
                    return cosS_sb, sinS_sb, cosE_sb, sinE_sb, transB_sb

                # ================= V projection (a-split, natural) ==========
                # Vb store: per J-chunk tile [128, (h,a,dh)=4096]
                vb_sb = []
                for jc in range(4):
                    vt = vb_pool.tile([128, H * BLK * DH], BFD, tag=f"vb{jc}")
                    vb_sb.append(vt)

                def evict_v(ps, a, c, jsl, cst_a):
                    """PSUM [128 j, 512 vcols] -> vb (flat copy + rotated+
                    translated rot half)."""
                    cn_v = cst_a[:, c, 0:128].rearrange("p (h i) -> p h i", h=8)
                    sn_v = cst_a[:, c, 128:256].rearrange("p (h i) -> p h i", h=8)
                    tn_v = cst_a[:, c, 256:512].rearrange(
                        "p (h i t) -> p h i t", h=8, i=16, t=2)
                    pvr = tmps_pool.tile([128, 512], BFD, tag="pvr")
                    pv = pvr.rearrange(
                        "p (h half i t) -> p h half i t", h=8, half=2, i=16, t=2)
                    psv = ps.rearrange(
                        "p (h half i t) -> p h half i t", h=8, half=2, i=16, t=2)
                    nc.scalar.copy(out=pv[:, :, 1], in_=psv[:, :, 1])
                    x0 = pv[:, :, 1, :, 0]
                    x1 = pv[:, :, 1, :, 1]
                    dst = vb_sb[c].rearrange(
                        "p (h a half i t) -> p h a half i t",
                        h=16, a=4, half=2, i=16, t=2)
                    hlo, hhi = jsl * 8, (jsl + 1) * 8
                    dflat = dst[:, hlo:hhi, a, 0]
                    de = dst[:, hlo:hhi, a, 1, :, 0]
                    do = dst[:, hlo:hhi, a, 1, :, 1]
                    nc.scalar.copy(out=dflat, in_=psv[:, :, 0])
                    t0 = tmps_pool.tile([128, 8, 16], BFD, tag="t0")
                    t1 = tmps_pool.tile([128, 8, 16], BFD, tag="t1")
                    t4 = tmps_pool.tile([128, 8, 16], BFD, tag="t4")
                    veng = nc.vector if (a * 4 + c) % 3 else nc.gpsimd
                    veng.tensor_tensor(t0, x0, cn_v, MULT)
                    veng.tensor_tensor(t1, x1, sn_v, MULT)
                    veng.tensor_tensor(t4, t0, t1, SUB)
                    # even_rot = x0 cos - x1 sin + c*trans_even
                    veng.tensor_tensor(de, t4, tn_v[:, :, :, 0], ADD)
                    t2 = tmps_pool.tile([128, 8, 16], BFD, tag="t2")
                    t3 = tmps_pool.tile([128, 8, 16], BFD, tag="t3")
                    t5 = tmps_pool.tile([128, 8, 16], BFD, tag="t5")
                    veng.tensor_tensor(t2, x0, sn_v, MULT)
                    veng.tensor_tensor(t3, x1, cn_v, MULT)
                    veng.tensor_tensor(t5, t2, t3, ADD)
                    veng.tensor_tensor(do, t5, tn_v[:, :, :, 1], ADD)

                def emit_v_phase():
                    for a in range(BLK):
                        cst_a = cn_pool.tile([128, 4, 512], BFD, tag="cstN",
                                             name="cst_a")
                        nc.sync.dma_start(out=cst_a, in_=cstN_d[b, a])
                        xv = [xT_sb[dk].rearrange(
                            "p (c j a) -> p c j a", c=4, j=128, a=4)
                            for dk in range(8)]
                        for c in range(4):
                            for jsl in range(2):
                                ps = ps_pool.tile([128, 512], FP32, tag="ps",
                                                  name="psv")
                                for dk in range(8):
                                    nc.tensor.matmul(
                                        ps, xv[dk][:, c, :, a],
                                        wv_sb[0][:, dk, jsl * 512:(jsl + 1) * 512],
                                        start=(dk == 0), stop=(dk == 7))
                                evict_v(ps, a, c, jsl, cst_a)

                # ================= Q/K pairs + attention ====================
                xa = [xT_sb[dk].rearrange("p (i a) -> p i a", a=4)
                      for dk in range(8)]
                coef_tiles = {}

                def emit_rot(which, qk_tiles):
                    # rotation in stacked layout (in-place, [128, 1024] ops)
                    cosS_sb, sinS_sb = coef_tiles["cosS"], coef_tiles["sinS"]
                    for hh in range(2):
                        ft = qk_tiles[(which, hh)]
                        shuf = tmpb_pool.tile([128, 2, NB], BFD, tag="shuf")
                        nc.vector.stream_shuffle(shuf, ft, swap_mask)
                        nc.vector.tensor_tensor(ft, ft, cosS_sb, MULT)
                        nc.vector.tensor_tensor(shuf, shuf, sinS_sb, MULT)
                        nc.vector.tensor_tensor(ft, ft, shuf, ADD)

                def emit_proj_w(c2, wi, which, wt, qk_tiles):
                    """Project one of q/k for chunk c2 (2 heads) into STACKED
                    tiles [(u,dh), (cpair, I)] per head, then rotate."""
                    st = {}
                    for hh in range(2):
                        ft = qk_pool.tile([128, 2, NB], BFD,
                                          tag=f"{which}S{hh}", name="ft")
                        st[hh] = ft
                        qk_tiles[(which, hh)] = ft
                    for a in range(4):
                        ps = ps_pool.tile([128, 512], FP32, tag="ps")
                        for dk in range(8):
                            nc.tensor.matmul(
                                ps, wt[:, dk, wi, :], xa[dk][:, :, a],
                                start=(dk == 0), stop=(dk == 7))
                        for hh in range(2):
                            nc.scalar.copy(
                                out=st[hh][(a % 2) * 64:(a % 2 + 1) * 64,
                                           a // 2, :],
                                in_=ps[hh * 64:(hh + 1) * 64, :])
                    emit_rot(which, qk_tiles)

                def emit_proj0_mm(wt, qk_tiles):
                    """proj(0), q AND k, dk-outer over 8 PSUM chains so the
                    batch-0 PE rides the xT DMA arrival. Rotation deferred
                    (cosS not yet loaded)."""
                    st = {}
                    for wi, which in ((0, "q"), (1, "k")):
                        for hh in range(2):
                            ft = qk_pool.tile([128, 2, NB], BFD,
                                              tag=f"{which}S{hh}", name="ft")
                            st[(wi, hh)] = ft
                            qk_tiles[(which, hh)] = ft
                    pss = [[ps_pool.tile([128, 512], FP32, tag="ps",
                                         name=f"psp{wi}{a}")
                            for a in range(4)] for wi in range(2)]
                    dk_order = [1, 0, 3, 2, 5, 4, 7, 6]  # xT arrival order
                    for di, dk in enumerate(dk_order):
                        for wi in range(2):
                            for a in range(4):
                                nc.tensor.matmul(
                                    pss[wi][a], wt[:, dk, wi, :],
                                    xa[dk][:, :, a],
                                    start=(di == 0), stop=(di == 7))
                    for wi in range(2):
                        for a in range(4):
                            for hh in range(2):
                                nc.scalar.copy(
                                    out=st[(wi, hh)][
                                        (a % 2) * 64:(a % 2 + 1) * 64,
                                        a // 2, :],
                                    in_=pss[wi][a][hh * 64:(hh + 1) * 64, :])

                def emit_sims(c2, qk_tiles):
                    """sims + exps for both heads; pre-adds on Pool."""
                    expts = {}
                    sums = {}
                    for hh in range(2):
                        for Jc in range(4):
                            sim_ps = ps_pool.tile([128, 512], FP32, tag="ps",
                                                  name="sim")
                            for cpair in range(2):
                                nc.tensor.matmul(
                                    sim_ps,
                                    qk_tiles[("k", hh)][
                                        :, cpair, Jc * 128:(Jc + 1) * 128],
                                    qk_tiles[("q", hh)][:, cpair, :],
                                    start=(cpair == 0), stop=(cpair == 1))
                            et = exp_pool.tile([128, 512], BFD, tag="expt")
                            nc.scalar.activation(
                                out=et, in_=sim_ps,
                                func=mybir.ActivationFunctionType.Exp,
                                scale=SCALE)
                            expts[(hh, Jc)] = et
                        s01 = sum_pool.tile([128, 512], BFD, tag="s01")
                        s23 = sum2_pool.tile([128, 512], BFD, tag="s23")
                        nc.gpsimd.tensor_tensor(
                            s01, expts[(hh, 0)], expts[(hh, 1)], ADD)
                        nc.gpsimd.tensor_tensor(
                            s23, expts[(hh, 2)], expts[(hh, 3)], ADD)
                        sums[hh] = (s01, s23)
                    return expts, sums

                def emit_attn_tail(c2, expts, sums, mid_hook=None):
                    cosE_sb = coef_tiles["cosE"]
                    sinE_sb = coef_tiles["sinE"]
                    transB_sb = coef_tiles["transB"]
                    mid_result = None
                    # PV + ACT eviction to SBUF (frees PSUM early); placed
                    # BEFORE the denominator matmuls so the slow Pool pre-add
                    # chain has PV's PE time to complete
                    pves = {}
                    for hh in range(2):
                        h = 2 * c2 + hh
                        for cp in range(2):
                            pv_ps = ps_pool.tile([128, 512], FP32, tag="ps",
                                                 name=f"pv{cp}")
                            for Jc in range(4):
                                lhsT = vb_sb[Jc].rearrange(
                                    "p (h a d) -> p h a d", h=16, a=4, d=64)[
                                        :, h, 2 * cp:2 * cp + 2, :]
                                nc.tensor.matmul(
                                    pv_ps, lhsT, expts[(hh, Jc)],
                                    start=(Jc == 0), stop=(Jc == 3))
                            pve = pve_pool.tile([128, 512], BFD, tag="pve")
                            nc.scalar.copy(out=pve, in_=pv_ps)
                            pves[(hh, cp)] = pve
                    if mid_hook is not None:
                        mid_result = mid_hook()
                    # denominator matmuls + Ln/exp reciprocal
                    rsums_t = {}
                    for hh in range(2):
                        sums_ps = ps_pool.tile([128, 512], FP32, tag="ps",
                                               name="sums")
                        nc.tensor.matmul(sums_ps, ones_sb, sums[hh][0],
                                         start=True, stop=False)
                        nc.tensor.matmul(sums_ps, ones_sb, sums[hh][1],
                                         start=False, stop=True)
                        nc.scalar.activation(
                            out=sums_ps, in_=sums_ps,
                            func=mybir.ActivationFunctionType.Ln)
                        rsums = tmpa_pool.tile([128, 512], BFD, tag="rsums")
                        nc.scalar.activation(
                            out=rsums, in_=sums_ps,
                            func=mybir.ActivationFunctionType.Exp, scale=-1.0)
                        rsums_t[hh] = rsums
                    # normalize + inverse transform (DVE, u2-mul on Pool)
                    for hh in range(2):
                        plo, phi = hh * 64, (hh + 1) * 64
                        for cp in range(2):
                            asb = tmpa_pool.tile([128, 512], BFD, tag="asb")
                            nc.vector.tensor_tensor(
                                asb, pves[(hh, cp)], rsums_t[hh], MULT)
                            nc.vector.tensor_tensor(
                                asb, asb, transB_sb[:, cp, :], SUB)
                            shf = tmpa_pool.tile([128, 512], BFD, tag="shf")
                            nc.vector.stream_shuffle(shf, asb, swap_mask)
                            nc.vector.tensor_tensor(
                                asb, asb, cosE_sb[:, cp, :], MULT)
                            nc.gpsimd.tensor_tensor(
                                shf, shf, sinE_sb[:, cp, :], MULT)
                            aov = ao_sb[c2].rearrange("p (a i) -> p a i", a=4)
                            for ap2 in range(2):
                                nc.vector.tensor_tensor(
                                    aov[plo:phi, 2 * cp + ap2, :],
                                    asb[ap2 * 64:(ap2 + 1) * 64, :],
                                    shf[ap2 * 64:(ap2 + 1) * 64, :],
                                    ADD)
                    return mid_result

                # ---- proj(0): combined q+k dk-outer emitted BEFORE the V
                # phase so the batch-0 PE rides the xT DMA arrival
                qk0 = {}
                emit_proj0_mm(wt0, qk0)
                emit_v_phase()
                cosS_sb, sinS_sb, cosE_sb, sinE_sb, transB_sb = load_coefs()
                coef_tiles.update(cosS=cosS_sb, sinS=sinS_sb, cosE=cosE_sb,
                                  sinE=sinE_sb, transB=transB_sb)
                if b == 0:
                    load_late_weights()
                ao_sb = []
                for c2 in range(8):
                    at = ao_pool.tile([128, N], BFD, tag=f"ao{c2}")
                    ao_sb.append(at)
                emit_rot("q", qk0)
                emit_rot("k", qk0)

                # -- output projection helpers (NWAVE warmup is emitted inside
                # the last attention iteration to pad its PE gaps)
                out_v = out_d[b].rearrange("(i a) e -> i a e", a=4)
                groups = [(a, cI, esl) for a in range(4) for cI in range(4)
                          for esl in range(2)]
                NWAVE = 2

                def outp_evict(ps, a, cI, esl):
                    oev = oev_pool.tile([128, 512], BFD, tag="oev")
                    nc.vector.tensor_tensor(
                        oev, ps, bout_sb[0][:, esl * 512:(esl + 1) * 512], ADD)
                    nc.sync.dma_start(
                        out=out_v[cI * 128:(cI + 1) * 128, a,
                                  esl * 512:(esl + 1) * 512],
                        in_=oev)

                def outp_warmup():
                    wave = []
                    for gi in range(NWAVE):
                        a, cI, esl = groups[gi]
                        ps = ps_pool.tile([128, 512], FP32, tag="ps",
                                          name=f"fw{gi}")
                        for jc in range(6):
                            nc.tensor.matmul(
                                ps,
                                ao_sb[jc][:, a * 512 + cI * 128:
                                          a * 512 + (cI + 1) * 128],
                                wout_sb[0][:, jc, esl * 512:(esl + 1) * 512],
                                start=(jc == 0), stop=False)
                        wave.append(ps)
                    return wave

                qk_prev = qk0
                prev_c2 = 0
                wave = None
                for c2 in range(1, 9):
                    qk_cur = {}
                    if c2 < 8:
                        wt = wqk_pool.tile([128, 8, 2, 128], BFD,
                                           tag="wqk", name="wt")
                        nc.sync.dma_start(out=wt, in_=wqkB_d[:, c2])
                        emit_proj_w(c2, 0, "q", wt, qk_cur)
                    expts, sums = emit_sims(prev_c2, qk_prev)
                    if c2 < 8:
                        emit_proj_w(c2, 1, "k", wt, qk_cur)
                        emit_attn_tail(prev_c2, expts, sums)
                    else:
                        wave = emit_attn_tail(prev_c2, expts, sums,
                                              mid_hook=outp_warmup)
                    qk_prev = qk_cur
                    prev_c2 = c2

                # ================= output projection ========================
                for gi in range(NWAVE):
                    a, cI, esl = groups[gi]
                    for jc in (6, 7):
                        nc.tensor.matmul(
                            wave[gi],
                            ao_sb[jc][:, a * 512 + cI * 128:
                                      a * 512 + (cI + 1) * 128],
                            wout_sb[0][:, jc, esl * 512:(esl + 1) * 512],
                            start=False, stop=(jc == 7))
                    outp_evict(wave[gi], a, cI, esl)
                for gi in range(NWAVE, len(groups)):
                    a, cI, esl = groups[gi]
                    ps = ps_pool.tile([128, 512], FP32, tag="ps")
                    for jc in range(8):
                        nc.tensor.matmul(
                            ps,
                            ao_sb[jc][:, a * 512 + cI * 128:
                                      a * 512 + (cI + 1) * 128],
                            wout_sb[0][:, jc, esl * 512:(esl + 1) * 512],
                            start=(jc == 0), stop=(jc == 7))
                    outp_evict(ps, a, cI, esl)
    _split_multi_waits(nc)
    return nc


def _host_prep(x, angles, trans, W_qkv, W_out, b_out, trans_coeff):
    """Build all per-core input arrays (layout/dtype staging + cos/sin coeffs)."""
    c = float(np.asarray(trans_coeff).reshape(-1)[0])
    cos = np.cos(angles).astype(np.float32)   # [B, N, 16]
    sin = np.sin(angles).astype(np.float32)

    xT = np.ascontiguousarray(x.transpose(0, 2, 1)).astype(BF16)       # [B, DIM, N]
    wqkvT = np.ascontiguousarray(np.asarray(W_qkv).T).astype(np.float32)  # [DIM, 3HDH]
    # wqkB[p, c2, dk, w, j] = wqkvT[dk*128+p, w*1024 + c2*128 + j]
    wqkB = np.ascontiguousarray(
        wqkvT[:, :2048].reshape(8, 128, 2, 8, 128)
        .transpose(1, 3, 0, 2, 4)).astype(BF16)
    # wvB[p, dk, j] = wqkvT[dk*128+p, 2048+j]
    wvB = np.ascontiguousarray(
        wqkvT[:, 2048:].reshape(8, 128, 1024).transpose(1, 0, 2)).astype(BF16)
    woutT = np.asarray(W_out).T.astype(np.float32)                     # [DIM, DIM]
    woutB = np.ascontiguousarray(
        woutT.reshape(8, 128, 1024).transpose(1, 0, 2)).astype(BF16)
    boutB = np.ascontiguousarray(
        np.broadcast_to(np.asarray(b_out)[None, :], (128, DIM))).astype(np.float32)

    dh = np.arange(DH)
    pair_idx = np.clip((dh - D_FLAT) // 2, 0, NPAIR - 1)               # [64]
    is_rot = dh >= D_FLAT
    is_odd = ((dh - D_FLAT) % 2 == 1) & is_rot

    I = np.arange(NB)

    # ---- cosS/sinS [B, 128, 2, NB]: rows = (u, dh); fwd rotation in stacked
    # layout: token t = 4I + 2*cpair + u
    sgn = np.where(is_rot, np.where(is_odd, 1.0, -1.0), 0.0)
    cosS = np.empty((B, 128, 2, NB), np.float32)
    sinS = np.empty((B, 128, 2, NB), np.float32)
    for cpair in range(2):
        for u in range(2):
            t_idx = 4 * I + 2 * cpair + u
            cc = cos[:, t_idx, :][:, :, pair_idx].transpose(0, 2, 1)   # [B,64,NB]
            ss = sin[:, t_idx, :][:, :, pair_idx].transpose(0, 2, 1)
            cosS[:, u * 64:(u + 1) * 64, cpair, :] = np.where(
                is_rot[None, :, None], cc, 1.0)
            sinS[:, u * 64:(u + 1) * 64, cpair, :] = ss * sgn[None, :, None]
    cosS = cosS.astype(BF16)
    sinS = sinS.astype(BF16)

    # ---- cstN [B, BLK, 128, 4, 512] for V: rows = J%128, c = J//128,
    # cols (h=8, i=16) x {cos, sin, c*trans}
    J = np.arange(NB)
    cstN = np.empty((B, BLK, NB, 512), np.float32)
    for a in range(BLK):
        t_idx = 4 * J + a
        cstN[:, a, :, 0:128] = np.tile(cos[:, t_idx, :], (1, 1, 8))
        cstN[:, a, :, 128:256] = np.tile(sin[:, t_idx, :], (1, 1, 8))
        cstN[:, a, :, 256:512] = np.tile(c * np.asarray(trans)[:, t_idx, :], (1, 1, 8))
    cstN = np.ascontiguousarray(
        cstN.reshape(B, BLK, 4, 128, 512).transpose(0, 1, 3, 2, 4)).astype(BF16)

    # ---- inverse coeffs [B, 128, 2, NB]: rows = (a2, dh); t = 4I + 2*cp + a2
    cosE = np.empty((B, 128, 2, NB), np.float32)
    sinE = np.empty((B, 128, 2, NB), np.float32)
    transB = np.zeros((B, 128, 2, NB), np.float32)
    sgnE = np.where(is_rot, np.where(is_odd, -1.0, 1.0), 0.0)
    for cp in range(2):
        for a2 in range(2):
            t_idx = 4 * I + 2 * cp + a2
            cc = cos[:, t_idx, :][:, :, pair_idx].transpose(0, 2, 1)   # [B,64,NB]
            ss = sin[:, t_idx, :][:, :, pair_idx].transpose(0, 2, 1)
            cosE[:, a2 * 64:(a2 + 1) * 64, cp, :] = np.where(
                is_rot[None, :, None], cc, 1.0)
            sinE[:, a2 * 64:(a2 + 1) * 64, cp, :] = ss * sgnE[None, :, None]
            tb = c * np.asarray(trans)[:, t_idx, :].transpose(0, 2, 1)  # [B,32,NB]
            transB[:, a2 * 64 + D_FLAT:(a2 + 1) * 64, cp, :] = tb
    cosE = cosE.astype(BF16)
    sinE = sinE.astype(BF16)

    return dict(xT=xT, wqkB=wqkB, wvB=wvB, woutB=woutB, boutB=boutB,
                cosS=cosS, sinS=sinS, cstN=cstN,
                cosE=cosE, sinE=sinE, transB=transB.astype(BF16))


def kernel(x, angles, trans, W_qkv, W_out, b_out, trans_coeff, _profile=False):
    x = np.asarray(x)
    angles = np.asarray(angles)
    trans = np.asarray(trans)
    arrs = _host_prep(x, angles, trans, W_qkv, W_out, b_out, trans_coeff)
    if "nc" not in _CACHE:
        _CACHE["nc"] = _build_nc()
    nc = _CACHE["nc"]

    in_maps = []
    for core in range(NCORES):
        bsl = slice(core * B2, (core + 1) * B2)
        m = dict(
            xT=np.ascontiguousarray(arrs["xT"][bsl]),
            wqkB=arrs["wqkB"], wvB=arrs["wvB"], woutB=arrs["woutB"],
            boutB=arrs["boutB"],
            cosS=np.ascontiguousarray(arrs["cosS"][bsl]),
            sinS=np.ascontiguousarray(arrs["sinS"][bsl]),
            cstN=np.ascontiguousarray(arrs["cstN"][bsl]),
            cosE=np.ascontiguousarray(arrs["cosE"][bsl]),
            sinE=np.ascontiguousarray(arrs["sinE"][bsl]),
            transB=np.ascontiguousarray(arrs["transB"][bsl]),
        )
        in_maps.append(m)

    res = run_bass_kernel_spmd(nc, in_maps, core_ids=list(range(NCORES)),
                               trace=_profile)
    out = np.concatenate([r["out"] for r in res.results], axis=0).astype(np.float32)
    if _profile:
        _CACHE["last_exec_time_ns"] = res.exec_time_ns
        _CACHE["last_trace"] = res.instructions_and_trace
    return out


# revision 108
# speedup vs baseline: 1.0078x; 1.0034x over previous
"""Trainium2 Bass kernel for nn_Attention_65317862638379.

Dense transformer block-attention with per-token geometric (rotation+translation)
transform. B=16, N=2048, DIM=1024, H=16, DH=64; attention over N/4=512 block
tokens of dim 256.

Sharding: data-parallel over batch, 2 batches per core, 8 cores, no collectives.
All matmuls run in bf16 on the PE (fp32 PSUM accumulation).

Layouts (per batch, per core):
  - Q/K: projection emitted per block-phase a (strided rhs) -> per-a PSUM
    [j=(head-pair,dh), I]; evicted with partition-offset copies into STACKED
    tiles qS/kS [(u,dh), I] per (head, a-pair) so the attention-score matmul
    contracts K=128 (2 steps) instead of K=64 (4 steps). Rotation applied
    in stacked layout via stream_shuffle (partition pair swap) + cosS/sinS.
  - V: a-split natural projection -> Vb [J, (h,a,dh)] block layout; fwd
    rotation on the free axis + translation. First (batch 0, a=0) group runs
    dk-outer over 8 PSUM banks so the PE pipelines with xT DMA arrival.
  - Attention sim[J, I] per head; softmax without max-subtraction; exp tiles
    pre-added pairwise on Pool so the partition-sum needs TWO ones-matmul
    accumulation steps (deps on two independent pool ops, not a chain).
  - Per-iteration emission [q-proj, sims(prev), k-proj, pv(prev), sums(prev)]
    keeps the static PE stream fed while ACT exps / pool pre-adds complete.
  - PV -> A [(a,dh), I]; normalize, inverse translate/rotate, write ao [j, t];
    final projection back to natural [t, e].
  - All weight/coefficient tensors are host-packed so each load is a single
    large DMA (HWDGE fixed cost is per-DMA).
"""

import numpy as np
import ml_dtypes

import concourse.bass as bass
import concourse.mybir as mybir
import concourse.tile as tile
from concourse.bass_utils import run_bass_kernel_spmd

BF16 = ml_dtypes.bfloat16

B, N, DIM, H, DH = 16, 2048, 1024, 16, 64
D_FLAT, D_ROT, NPAIR = 32, 32, 16
BLK = 4
NB = N // BLK          # 512 block tokens
DB = DH * BLK          # 256 block dim
NCORES = 8
B2 = B // NCORES       # batches per core
SCALE = float((DH * BLK) ** -0.5)  # 1/16, TAU=1.0

FP32 = mybir.dt.float32
BFD = mybir.dt.bfloat16

MULT = mybir.AluOpType.mult
ADD = mybir.AluOpType.add
SUB = mybir.AluOpType.subtract

_CACHE = {}


def _split_multi_waits(nc):
    """walrus codegen only supports one sync-wait per instruction; hoist
    extra waits onto preceding same-engine NoOps."""
    cnt = 0
    for f in nc.m.functions:
        for blk in f.blocks:
            insts = blk.instructions
            out = []
            for inst in insts:
                si = inst.sync_info
                if si is not None and si.on_wait and len(si.on_wait) > 1:
                    waits = list(si.on_wait)
                    for w in waits[:-1]:
                        cnt += 1
                        nop = mybir.InstNoOp(name=f"WSPLIT-{cnt}", ins=[], outs=[])
                        nop.engine = inst.engine
                        nop.sync_info = mybir.SyncInfo(on_wait=[w], on_update=[])
                        out.append(nop)
                    inst.sync_info = mybir.SyncInfo(
                        on_wait=[waits[-1]], on_update=list(si.on_update))
                out.append(inst)
            blk.instructions = out
    return cnt


def _build_nc():
    """Build the Bass graph (SPMD; same NEFF on all 8 cores)."""
    nc = bass.Bass(target_bir_lowering=False)

    # ---------------- DRAM parameters (per-core shapes) ----------------
    # host-packed layouts: leading dim 128 = SBUF partition
    xT_d = nc.dram_tensor("xT", [B2, DIM, N], BFD, kind="ExternalInput")
    wqkB_d = nc.dram_tensor("wqkB", [128, 8, 8, 2, 128], BFD, kind="ExternalInput")
    wvB_d = nc.dram_tensor("wvB", [128, 8, 1024], BFD, kind="ExternalInput")
    woutB_d = nc.dram_tensor("woutB", [128, 8, 1024], BFD, kind="ExternalInput")
    boutB_d = nc.dram_tensor("boutB", [128, DIM], FP32, kind="ExternalInput")
    cosS_d = nc.dram_tensor("cosS", [B2, 128, 2, NB], BFD, kind="ExternalInput")
    sinS_d = nc.dram_tensor("sinS", [B2, 128, 2, NB], BFD, kind="ExternalInput")
    cstN_d = nc.dram_tensor("cstN", [B2, BLK, 128, 4, 512], BFD, kind="ExternalInput")
    cosE_d = nc.dram_tensor("cosE", [B2, 128, 2, NB], BFD, kind="ExternalInput")
    sinE_d = nc.dram_tensor("sinE", [B2, 128, 2, NB], BFD, kind="ExternalInput")
    transB_d = nc.dram_tensor("transB", [B2, 128, 2, NB], BFD, kind="ExternalInput")

    out_d = nc.dram_tensor("out", [B2, N, DIM], BFD, kind="ExternalOutput")

    swap_mask = []
    for i in range(16):
        swap_mask += [2 * i + 1, 2 * i]

    from contextlib import ExitStack
    with ExitStack() as ctx:
        tc = ctx.enter_context(tile.TileContext(nc))
        ep = ctx.enter_context
        consts = ep(tc.tile_pool(name="consts", bufs=1))
        xT_pool = ep(tc.tile_pool(name="xT", bufs=1))
        wv_pool = ep(tc.tile_pool(name="wv", bufs=1))
        wqk_pool = ep(tc.tile_pool(name="wqk", bufs=2))
        wout_pool = ep(tc.tile_pool(name="wout", bufs=1))
        vb_pool = ep(tc.tile_pool(name="vb", bufs=1))
        qk_pool = ep(tc.tile_pool(name="qk", bufs=2))
        ao_pool = ep(tc.tile_pool(name="ao", bufs=1))
        coef_pool = ep(tc.tile_pool(name="coefs", bufs=1))
        cn_pool = ep(tc.tile_pool(name="cn", bufs=2))
        exp_pool = ep(tc.tile_pool(name="expt", bufs=8))
        sum_pool = ep(tc.tile_pool(name="sums", bufs=2))
        sum2_pool = ep(tc.tile_pool(name="sums2", bufs=2))
        tmps_pool = ep(tc.tile_pool(name="tmps", bufs=2))
        tmpb_pool = ep(tc.tile_pool(name="tmpb", bufs=2))
        tmpa_pool = ep(tc.tile_pool(name="tmpa", bufs=2))
        pve_pool = ep(tc.tile_pool(name="pve", bufs=3))
        oev_pool = ep(tc.tile_pool(name="oev", bufs=3))
        ps_pool = ep(tc.tile_pool(name="ps", bufs=8, space="PSUM"))
        if True:
            wv_sb = {}
            wqk_sb = {}
            wout_sb = {}
            bout_sb = {}
            ones_sb = consts.tile([128, 128], BFD)
            nc.vector.memset(ones_sb, 1.0)

            def load_first_weights():
                # wv in two half-DMAs on the sync queue, AFTER the xT loads
                # (DMA engines are serial: wv must not delay xT0)
                wv_sb[0] = wv_pool.tile([128, 8, 1024], BFD, tag="wv", name="wv")
                nc.sync.dma_start(out=wv_sb[0][:, :, 0:512],
                                  in_=wvB_d[:, :, 0:512])
                nc.sync.dma_start(out=wv_sb[0][:, :, 512:1024],
                                  in_=wvB_d[:, :, 512:1024])

            def load_late_weights():
                wout_sb[0] = wout_pool.tile([128, 8, 1024], BFD, tag="wout", name="wout")
                nc.sync.dma_start(out=wout_sb[0], in_=woutB_d[:, :, :])
                bout_sb[0] = consts.tile([128, DIM], FP32, tag="bout", name="bout")
                nc.sync.dma_start(out=bout_sb[0], in_=boutB_d[:, :])

            for b in range(B2):
                # wqk chunk-0 weights first: proj(0) dk-outer rides xT arrival
                wt0 = wqk_pool.tile([128, 8, 2, 128], BFD, tag="wqk",
                                    name="wt0")
                # split: tiny first piece (dk 0-1) unblocks the first
                # dk-outer rounds and lets xT1 transfer sooner
                nc.sync.dma_start(out=wt0[:, 0:2], in_=wqkB_d[:, 0, 0:2])
                nc.sync.dma_start(out=wt0[:, 2:8], in_=wqkB_d[:, 0, 2:8])
                # ---- load xT for this batch: 8 d-chunk tiles [128, 2048] ----
                xT_sb = []
                for dk in range(8):
                    t = xT_pool.tile([128, N], BFD, tag=f"xT{dk}", name=f"xT{dk}")
                    deng = [nc.sync, nc.gpsimd][dk % 2]
                    deng.dma_start(
                        out=t, in_=xT_d[b, dk * 128:(dk + 1) * 128, :])
                    xT_sb.append(t)
                if b == 0:
                    load_first_weights()

                # ---- per-batch coefficient tiles ----
                def load_coefs():
                    cosS_sb = coef_pool.tile([128, 2, NB], BFD, tag="cosS", name="cosS")
                    sinS_sb = coef_pool.tile([128, 2, NB], BFD, tag="sinS", name="sinS")
                    nc.sync.dma_start(out=cosS_sb, in_=cosS_d[b])
                    nc.sync.dma_start(out=sinS_sb, in_=sinS_d[b])
                    cosE_sb = coef_pool.tile([128, 2, NB], BFD, tag="cosE", name="cosE")
                    sinE_sb = coef_pool.tile([128, 2, NB], BFD, tag="sinE", name="sinE")
                    transB_sb = coef_pool.tile([128, 2, NB], BFD, tag="transB", name="transB")
                    nc.sync.dma_start(out=cosE_sb, in_=cosE_d[b])
                    nc.sync.dma_start(out=sinE_sb, in_=sinE_d[b])
                    nc.sync.dma_start(out=transB_sb, in_=transB_d[b])
                    return cosS_sb, sinS_sb, cosE_sb, sinE_sb, transB_sb

                # ================= V projection (a-split, natural) ==========
                # Vb store: per J-chunk tile [128, (h,a,dh)=4096]
                vb_sb = []
                for jc in range(4):
                    vt = vb_pool.tile([128, H * BLK * DH], BFD, tag=f"vb{jc}")
                    vb_sb.append(vt)

                def evict_v(ps, a, c, jsl, cst_a):
                    """PSUM [128 j, 512 vcols] -> vb (flat copy + rotated+
                    translated rot half)."""
                    cn_v = cst_a[:, c, 0:128].rearrange("p (h i) -> p h i", h=8)
                    sn_v = cst_a[:, c, 128:256].rearrange("p (h i) -> p h i", h=8)
                    tn_v = cst_a[:, c, 256:512].rearrange(
                        "p (h i t) -> p h i t", h=8, i=16, t=2)
                    pvr = tmps_pool.tile([128, 512], BFD, tag="pvr")
                    pv = pvr.rearrange(
                        "p (h half i t) -> p h half i t", h=8, half=2, i=16, t=2)
                    psv = ps.rearrange(
                        "p (h half i t) -> p h half i t", h=8, half=2, i=16, t=2)
                    nc.scalar.copy(out=pv[:, :, 1], in_=psv[:, :, 1])
                    x0 = pv[:, :, 1, :, 0]
                    x1 = pv[:, :, 1, :, 1]
                    dst = vb_sb[c].rearrange(
                        "p (h a half i t) -> p h a half i t",
                        h=16, a=4, half=2, i=16, t=2)
                    hlo, hhi = jsl * 8, (jsl + 1) * 8
                    dflat = dst[:, hlo:hhi, a, 0]
                    de = dst[:, hlo:hhi, a, 1, :, 0]
                    do = dst[:, hlo:hhi, a, 1, :, 1]
                    nc.scalar.copy(out=dflat, in_=psv[:, :, 0])
                    t0 = tmps_pool.tile([128, 8, 16], BFD, tag="t0")
                    t1 = tmps_pool.tile([128, 8, 16], BFD, tag="t1")
                    t4 = tmps_pool.tile([128, 8, 16], BFD, tag="t4")
                    veng = nc.vector if (a * 4 + c) % 3 else nc.gpsimd
                    veng.tensor_tensor(t0, x0, cn_v, MULT)
                    veng.tensor_tensor(t1, x1, sn_v, MULT)
                    veng.tensor_tensor(t4, t0, t1, SUB)
                    # even_rot = x0 cos - x1 sin + c*trans_even
                    veng.tensor_tensor(de, t4, tn_v[:, :, :, 0], ADD)
                    t2 = tmps_pool.tile([128, 8, 16], BFD, tag="t2")
                    t3 = tmps_pool.tile([128, 8, 16], BFD, tag="t3")
                    t5 = tmps_pool.tile([128, 8, 16], BFD, tag="t5")
                    veng.tensor_tensor(t2, x0, sn_v, MULT)
                    veng.tensor_tensor(t3, x1, cn_v, MULT)
                    veng.tensor_tensor(t5, t2, t3, ADD)
                    veng.tensor_tensor(do, t5, tn_v[:, :, :, 1], ADD)

                def emit_v_phase():
                    for a in range(BLK):
                        cst_a = cn_pool.tile([128, 4, 512], BFD, tag="cstN",
                                             name="cst_a")
                        nc.sync.dma_start(out=cst_a, in_=cstN_d[b, a])
                        xv = [xT_sb[dk].rearrange(
                            "p (c j a) -> p c j a", c=4, j=128, a=4)
                            for dk in range(8)]
                        for c in range(4):
                            for jsl in range(2):
                                ps = ps_pool.tile([128, 512], FP32, tag="ps",
                                                  name="psv")
                                for dk in range(8):
                                    nc.tensor.matmul(
                                        ps, xv[dk][:, c, :, a],
                                        wv_sb[0][:, dk, jsl * 512:(jsl + 1) * 512],
                                        start=(dk == 0), stop=(dk == 7))
                                evict_v(ps, a, c, jsl, cst_a)

                # ================= Q/K pairs + attention ====================
                xa = [xT_sb[dk].rearrange("p (i a) -> p i a", a=4)
                      for dk in range(8)]
                coef_tiles = {}

                def emit_rot(which, qk_tiles):
                    # rotation in stacked layout (in-place, [128, 1024] ops)
                    cosS_sb, sinS_sb = coef_tiles["cosS"], coef_tiles["sinS"]
                    for hh in range(2):
                        ft = qk_tiles[(which, hh)]
                        shuf = tmpb_pool.tile([128, 2, NB], BFD, tag="shuf")
                        nc.vector.stream_shuffle(shuf, ft, swap_mask)
                        nc.vector.tensor_tensor(ft, ft, cosS_sb, MULT)
                        nc.vector.tensor_tensor(shuf, shuf, sinS_sb, MULT)
                        nc.vector.tensor_tensor(ft, ft, shuf, ADD)

                def emit_proj_w(c2, wi, which, wt, qk_tiles):
                    """Project one of q/k for chunk c2 (2 heads) into STACKED
                    tiles [(u,dh), (cpair, I)] per head, then rotate."""
                    st = {}
                    for hh in range(2):
                        ft = qk_pool.tile([128, 2, NB], BFD,
                                          tag=f"{which}S{hh}", name="ft")
                        st[hh] = ft
                        qk_tiles[(which, hh)] = ft
                    for a in range(4):
                        ps = ps_pool.tile([128, 512], FP32, tag="ps")
                        for dk in range(8):
                            nc.tensor.matmul(
                                ps, wt[:, dk, wi, :], xa[dk][:, :, a],
                                start=(dk == 0), stop=(dk == 7))
                        for hh in range(2):
                            nc.scalar.copy(
                                out=st[hh][(a % 2) * 64:(a % 2 + 1) * 64,
                                           a // 2, :],
                                in_=ps[hh * 64:(hh + 1) * 64, :])
                    emit_rot(which, qk_tiles)

                def emit_proj0_mm(wt, qk_tiles):
                    """proj(0), q AND k, dk-outer over 8 PSUM chains so the
                    batch-0 PE rides the xT DMA arrival. Rotation deferred
                    (cosS not yet loaded)."""
                    st = {}
                    for wi, which in ((0, "q"), (1, "k")):
                        for hh in range(2):
                            ft = qk_pool.tile([128, 2, NB], BFD,
                                              tag=f"{which}S{hh}", name="ft")
                            st[(wi, hh)] = ft
                            qk_tiles[(which, hh)] = ft
                    pss = [[ps_pool.tile([128, 512], FP32, tag="ps",
                                         name=f"psp{wi}{a}")
                            for a in range(4)] for wi in range(2)]
                    dk_order = [1, 0, 3, 2, 5, 4, 7, 6]  # xT arrival order
                    for di, dk in enumerate(dk_order):
                        for wi in range(2):
                            for a in range(4):
                                nc.tensor.matmul(
                                    pss[wi][a], wt[:, dk, wi, :],
                                    xa[dk][:, :, a],
                                    start=(di == 0), stop=(di == 7))
                    for wi in range(2):
                        for a in range(4):
                            for hh in range(2):
                                nc.scalar.copy(
                                    out=st[(wi, hh)][
                                        (a % 2) * 64:(a % 2 + 1) * 64,
                                        a // 2, :],
                                    in_=pss[wi][a][hh * 64:(hh + 1) * 64, :])

                def emit_sims(c2, qk_tiles):
                    """sims + exps for both heads; pre-adds on Pool."""
                    expts = {}
                    sums = {}
                    for hh in range(2):
                        for Jc in range(4):
                            sim_ps = ps_pool.tile([128, 512], FP32, tag="ps",
                                                  name="sim")
                            for cpair in range(2):
                                nc.tensor.matmul(
                                    sim_ps,
                                    qk_tiles[("k", hh)][
                                        :, cpair, Jc * 128:(Jc + 1) * 128],
                                    qk_tiles[("q", hh)][:, cpair, :],
                                    start=(cpair == 0), stop=(cpair == 1))
                            et = exp_pool.tile([128, 512], BFD, tag="expt")
                            nc.scalar.activation(
                                out=et, in_=sim_ps,
                                func=mybir.ActivationFunctionType.Exp,
                                scale=SCALE)
                            expts[(hh, Jc)] = et
                        s01 = sum_pool.tile([128, 512], BFD, tag="s01")
                        s23 = sum2_pool.tile([128, 512], BFD, tag="s23")
                        nc.gpsimd.tensor_tensor(
                            s01, expts[(hh, 0)], expts[(hh, 1)], ADD)
                        nc.gpsimd.tensor_tensor(
                            s23, expts[(hh, 2)], expts[(hh, 3)], ADD)
                        sums[hh] = (s01, s23)
                    return expts, sums

                def emit_attn_tail(c2, expts, sums, mid_hook=None):
                    cosE_sb = coef_tiles["cosE"]
                    sinE_sb = coef_tiles["sinE"]
                    transB_sb = coef_tiles["transB"]
                    mid_result = None
                    # PV + ACT eviction to SBUF (frees PSUM early); placed
                    # BEFORE the denominator matmuls so the slow Pool pre-add
                    # chain has PV's PE time to complete
                    pves = {}
                    for hh in range(2):
                        h = 2 * c2 + hh
                        for cp in range(2):
                            pv_ps = ps_pool.tile([128, 512], FP32, tag="ps",
                                                 name=f"pv{cp}")
                            for Jc in range(4):
                                lhsT = vb_sb[Jc].rearrange(
                                    "p (h a d) -> p h a d", h=16, a=4, d=64)[
                                        :, h, 2 * cp:2 * cp + 2, :]
                                nc.tensor.matmul(
                                    pv_ps, lhsT, expts[(hh, Jc)],
                                    start=(Jc == 0), stop=(Jc == 3))
                            pve = pve_pool.tile([128, 512], BFD, tag="pve")
                            nc.scalar.copy(out=pve, in_=pv_ps)
                            pves[(hh, cp)] = pve
                    if mid_hook is not None:
                        mid_result = mid_hook()
                    # denominator matmuls + Ln/exp reciprocal
                    rsums_t = {}
                    for hh in range(2):
                        sums_ps = ps_pool.tile([128, 512], FP32, tag="ps",
                                               name="sums")
                        nc.tensor.matmul(sums_ps, ones_sb, sums[hh][0],
                                         start=True, stop=False)
                        nc.tensor.matmul(sums_ps, ones_sb, sums[hh][1],
                                         start=False, stop=True)
                        nc.scalar.activation(
                            out=sums_ps, in_=sums_ps,
                            func=mybir.ActivationFunctionType.Ln)
                        rsums = tmpa_pool.tile([128, 512], BFD, tag="rsums")
                        nc.scalar.activation(
                            out=rsums, in_=sums_ps,
                            func=mybir.ActivationFunctionType.Exp, scale=-1.0)
                        rsums_t[hh] = rsums
                    # normalize + inverse transform (DVE, u2-mul on Pool)
                    for hh in range(2):
                        plo, phi = hh * 64, (hh + 1) * 64
                        for cp in range(2):
                            asb = tmpa_pool.tile([128, 512], BFD, tag="asb")
                            nc.vector.tensor_tensor(
                                asb, pves[(hh, cp)], rsums_t[hh], MULT)
                            nc.vector.tensor_tensor(
                                asb, asb, transB_sb[:, cp, :], SUB)
                            shf = tmpa_pool.tile([128, 512], BFD, tag="shf")
                            nc.vector.stream_shuffle(shf, asb, swap_mask)
                            nc.vector.tensor_tensor(
                                asb, asb, cosE_sb[:, cp, :], MULT)
                            nc.gpsimd.tensor_tensor(
                                shf, shf, sinE_sb[:, cp, :], MULT)
                            aov = ao_sb[c2].rearrange("p (a i) -> p a i", a=4)
                            for ap2 in range(2):
                                nc.vector.tensor_tensor(
                                    aov[plo:phi, 2 * cp + ap2, :],
                                    asb[ap2 * 64:(ap2 + 1) * 64, :],
                                    shf[ap2 * 64:(ap2 + 1) * 64, :],
                                    ADD)
                    return mid_result

                # ---- proj(0): combined q+k dk-outer emitted BEFORE the V
                # phase so the batch-0 PE rides the xT DMA arrival
                qk0 = {}
                emit_proj0_mm(wt0, qk0)
                emit_v_phase()
                cosS_sb, sinS_sb, cosE_sb, sinE_sb, transB_sb = load_coefs()
                coef_tiles.update(cosS=cosS_sb, sinS=sinS_sb, cosE=cosE_sb,
                                  sinE=sinE_sb, transB=transB_sb)
                if b == 0:
                    load_late_weights()
                ao_sb = []
                for c2 in range(8):
                    at = ao_pool.tile([128, N], BFD, tag=f"ao{c2}")
                    ao_sb.append(at)
                emit_rot("q", qk0)
                emit_rot("k", qk0)

                # -- output projection helpers (NWAVE warmup is emitted inside
                # the last attention iteration to pad its PE gaps)
                out_v = out_d[b].rearrange("(i a) e -> i a e", a=4)
                groups = [(a, cI, esl) for a in range(4) for cI in range(4)
                          for esl in range(2)]
                NWAVE = 2

                def outp_evict(ps, a, cI, esl):
                    oev = oev_pool.tile([128, 512], BFD, tag="oev")
                    nc.vector.tensor_tensor(
                        oev, ps, bout_sb[0][:, esl * 512:(esl + 1) * 512], ADD)
                    nc.sync.dma_start(
                        out=out_v[cI * 128:(cI + 1) * 128, a,
                                  esl * 512:(esl + 1) * 512],
                        in_=oev)

                def outp_warmup():
                    wave = []
                    for gi in range(NWAVE):
                        a, cI, esl = groups[gi]
                        ps = ps_pool.tile([128, 512], FP32, tag="ps",
                                          name=f"fw{gi}")
                        for jc in range(6):
                            nc.tensor.matmul(
                                ps,
                                ao_sb[jc][:, a * 512 + cI * 128:
                                          a * 512 + (cI + 1) * 128],
                                wout_sb[0][:, jc, esl * 512:(esl + 1) * 512],
                                start=(jc == 0), stop=False)
                        wave.append(ps)
                    return wave

                qk_prev = qk0
                prev_c2 = 0
                wave = None
                for c2 in range(1, 9):
                    qk_cur = {}
                    if c2 < 8:
                        wt = wqk_pool.tile([128, 8, 2, 128], BFD,
                                           tag="wqk", name="wt")
                        nc.sync.dma_start(out=wt, in_=wqkB_d[:, c2])
                        emit_proj_w(c2, 0, "q", wt, qk_cur)
                    expts, sums = emit_sims(prev_c2, qk_prev)
                    if c2 < 8:
                        emit_proj_w(c2, 1, "k", wt, qk_cur)
                        emit_attn_tail(prev_c2, expts, sums)
                    else:
                        wave = emit_attn_tail(prev_c2, expts, sums,
                                              mid_hook=outp_warmup)
                    qk_prev = qk_cur
                    prev_c2 = c2

                # ================= output projection ========================
                for gi in range(NWAVE):
                    a, cI, esl = groups[gi]
                    for jc in (6, 7):
                        nc.tensor.matmul(
                            wave[gi],
                            ao_sb[jc][:, a * 512 + cI * 128:
                                      a * 512 + (cI + 1) * 128],
                            wout_sb[0][:, jc, esl * 512:(esl + 1) * 512],
                            start=False, stop=(jc == 7))
                    outp_evict(wave[gi], a, cI, esl)
                for gi in range(NWAVE, len(groups)):
                    a, cI, esl = groups[gi]
                    ps = ps_pool.tile([128, 512], FP32, tag="ps")
                    for jc in range(8):
                        nc.tensor.matmul(
                            ps,
                            ao_sb[jc][:, a * 512 + cI * 128:
                                      a * 512 + (cI + 1) * 128],
                            wout_sb[0][:, jc, esl * 512:(esl + 1) * 512],
                            start=(jc == 0), stop=(jc == 7))
                    outp_evict(ps, a, cI, esl)
    _split_multi_waits(nc)
    return nc


def _host_prep(x, angles, trans, W_qkv, W_out, b_out, trans_coeff):
    """Build all per-core input arrays (layout/dtype staging + cos/sin coeffs)."""
    c = float(np.asarray(trans_coeff).reshape(-1)[0])
    cos = np.cos(angles).astype(np.float32)   # [B, N, 16]
    sin = np.sin(angles).astype(np.float32)

    xT = np.ascontiguousarray(x.transpose(0, 2, 1)).astype(BF16)       # [B, DIM, N]
    wqkvT = np.ascontiguousarray(np.asarray(W_qkv).T).astype(np.float32)  # [DIM, 3HDH]
    # wqkB[p, c2, dk, w, j] = wqkvT[dk*128+p, w*1024 + c2*128 + j]
    wqkB = np.ascontiguousarray(
        wqkvT[:, :2048].reshape(8, 128, 2, 8, 128)
        .transpose(1, 3, 0, 2, 4)).astype(BF16)
    # wvB[p, dk, j] = wqkvT[dk*128+p, 2048+j]
    wvB = np.ascontiguousarray(
        wqkvT[:, 2048:].reshape(8, 128, 1024).transpose(1, 0, 2)).astype(BF16)
    woutT = np.asarray(W_out).T.astype(np.float32)                     # [DIM, DIM]
    woutB = np.ascontiguousarray(
        woutT.reshape(8, 128, 1024).transpose(1, 0, 2)).astype(BF16)
    boutB = np.ascontiguousarray(
        np.broadcast_to(np.asarray(b_out)[None, :], (128, DIM))).astype(np.float32)

    dh = np.arange(DH)
    pair_idx = np.clip((dh - D_FLAT) // 2, 0, NPAIR - 1)               # [64]
    is_rot = dh >= D_FLAT
    is_odd = ((dh - D_FLAT) % 2 == 1) & is_rot

    I = np.arange(NB)

    # ---- cosS/sinS [B, 128, 2, NB]: rows = (u, dh); fwd rotation in stacked
    # layout: token t = 4I + 2*cpair + u
    sgn = np.where(is_rot, np.where(is_odd, 1.0, -1.0), 0.0)
    cosS = np.empty((B, 128, 2, NB), np.float32)
    sinS = np.empty((B, 128, 2, NB), np.float32)
    for cpair in range(2):
        for u in range(2):
            t_idx = 4 * I + 2 * cpair + u
            cc = cos[:, t_idx, :][:, :, pair_idx].transpose(0, 2, 1)   # [B,64,NB]
            ss = sin[:, t_idx, :][:, :, pair_idx].transpose(0, 2, 1)
            cosS[:, u * 64:(u + 1) * 64, cpair, :] = np.where(
                is_rot[None, :, None], cc, 1.0)
            sinS[:, u * 64:(u + 1) * 64, cpair, :] = ss * sgn[None, :, None]
    cosS = cosS.astype(BF16)
    sinS = sinS.astype(BF16)

    # ---- cstN [B, BLK, 128, 4, 512] for V: rows = J%128, c = J//128,
    # cols (h=8, i=16) x {cos, sin, c*trans}
    J = np.arange(NB)
    cstN = np.empty((B, BLK, NB, 512), np.float32)
    for a in range(BLK):
        t_idx = 4 * J + a
        cstN[:, a, :, 0:128] = np.tile(cos[:, t_idx, :], (1, 1, 8))
        cstN[:, a, :, 128:256] = np.tile(sin[:, t_idx, :], (1, 1, 8))
        cstN[:, a, :, 256:512] = np.tile(c * np.asarray(trans)[:, t_idx, :], (1, 1, 8))
    cstN = np.ascontiguousarray(
        cstN.reshape(B, BLK, 4, 128, 512).transpose(0, 1, 3, 2, 4)).astype(BF16)

    # ---- inverse coeffs [B, 128, 2, NB]: rows = (a2, dh); t = 4I + 2*cp + a2
    cosE = np.empty((B, 128, 2, NB), np.float32)
    sinE = np.empty((B, 128, 2, NB), np.float32)
    transB = np.zeros((B, 128, 2, NB), np.float32)
    sgnE = np.where(is_rot, np.where(is_odd, -1.0, 1.0), 0.0)
    for cp in range(2):
        for a2 in range(2):
            t_idx = 4 * I + 2 * cp + a2
            cc = cos[:, t_idx, :][:, :, pair_idx].transpose(0, 2, 1)   # [B,64,NB]
            ss = sin[:, t_idx, :][:, :, pair_idx].transpose(0, 2, 1)
            cosE[:, a2 * 64:(a2 + 1) * 64, cp, :] = np.where(
                is_rot[None, :, None], cc, 1.0)
            sinE[:, a2 * 64:(a2 + 1) * 64, cp, :] = ss * sgnE[None, :, None]
            tb = c * np.asarray(trans)[:, t_idx, :].transpose(0, 2, 1)  # [B,32,NB]
            transB[:, a2 * 64 + D_FLAT:(a2 + 1) * 64, cp, :] = tb
    cosE = cosE.astype(BF16)
    sinE = sinE.astype(BF16)

    return dict(xT=xT, wqkB=wqkB, wvB=wvB, woutB=woutB, boutB=boutB,
                cosS=cosS, sinS=sinS, cstN=cstN,
                cosE=cosE, sinE=sinE, transB=transB.astype(BF16))


def kernel(x, angles, trans, W_qkv, W_out, b_out, trans_coeff, _profile=False):
    x = np.asarray(x)
    angles = np.asarray(angles)
    trans = np.asarray(trans)
    arrs = _host_prep(x, angles, trans, W_qkv, W_out, b_out, trans_coeff)
    if "nc" not in _CACHE:
        _CACHE["nc"] = _build_nc()
    nc = _CACHE["nc"]

    in_maps = []
    for core in range(NCORES):
        bsl = slice(core * B2, (core + 1) * B2)
        m = dict(
            xT=np.ascontiguousarray(arrs["xT"][bsl]),
            wqkB=arrs["wqkB"], wvB=arrs["wvB"], woutB=arrs["woutB"],
            boutB=arrs["boutB"],
            cosS=np.ascontiguousarray(arrs["cosS"][bsl]),
            sinS=np.ascontiguousarray(arrs["sinS"][bsl]),
            cstN=np.ascontiguousarray(arrs["cstN"][bsl]),
            cosE=np.ascontiguousarray(arrs["cosE"][bsl]),
            sinE=np.ascontiguousarray(arrs["sinE"][bsl]),
            transB=np.ascontiguousarray(arrs["transB"][bsl]),
        )
        in_maps.append(m)

    res = run_bass_kernel_spmd(nc, in_maps, core_ids=list(range(NCORES)),
                               trace=_profile)
    out = np.concatenate([r["out"] for r in res.results], axis=0).astype(np.float32)
    if _profile:
        _CACHE["last_exec_time_ns"] = res.exec_time_ns
        _CACHE["last_trace"] = res.instructions_and_trace
    return out
